# revision 35
# baseline (speedup 1.0000x reference)
"""MoE routing kernel for Trainium2 (8 NeuronCores, data-parallel over tokens).

Computation (matches the reference exactly):
    scores = x @ gate_w.T                 [N, E]   fp32 (exact gating!)
    top4 mask per token, weight = top-1 score for all selected experts
    hs = (x * top1) @ sum_{e in top4} expert_w[e].T     (bf16 GEMMs)
    out = relu(hs)^2 @ out_w.T

Sharding: tokens split 1024/core (no collectives). Weights are replicated,
host-side pre-transposed to contraction-major layout and cast to bf16
(gate weights stay fp32 - the top-k selection is discrete and needs exact
scores; bf16 scores would flip expert selections).

Device pipeline per core:
  1. PE fp32 transpose of x tiles -> xT; fp32 gate GEMM -> scores [t,8]
  2. DVE max8 -> sorted scores; mask = (s >= s[3]); x16w = bf16(x * s[0])
  3. "z-trick": for each (token-tile, expert), one bf16 matmul with a
     mask-scaled identity as moving operand produces the masked transpose
     z_e = (x*top1*mask_e)^T. This lets the main expert GEMM accumulate
     over experts AND the contraction dim entirely inside PSUM.
  4. main GEMM: hs^T[i,t] += ewT[e][d,i].T @ z_e[d,t] over (d,e)
  5. relu^2 on ScalarE (fp32 relu, square with bf16 cast)
  6. out projection: out[t,:] = relu2(hs)[t,:] @ out_w.T
"""

import numpy as np
import ml_dtypes

_CACHE = {}

P = 128
T, D, E, I, DO = 1024, 1024, 8, 2048, 1024
TT, DD, II = T // P, D // P, I // P          # 8, 8, 16
NCH = 2                                      # token chunks per core
TPC = TT // NCH                              # t-tiles per chunk (4)
TC = TPC * P                                 # tokens per chunk (512)
NCORES = 8


def _split_sync_waits(nc):
    """walrus in this container caps sync waits per instruction (and rejects
    any wait on Drain). Move excess waits onto injected same-engine NOPs
    placed immediately before the instruction - the engine blocks on the
    nops' waits first, so the ordering semantics are identical."""
    from concourse import mybir

    uid = 0
    for bb in nc.m.functions[0].blocks:
        insts = bb.instructions
        new = []
        changed = False
        for inst in insts:
            si = getattr(inst, "sync_info", None)
            waits = list(si.on_wait) if si is not None and si.on_wait else []
            keep = 0 if isinstance(inst, mybir.InstDrain) else 1
            if len(waits) > keep:
                moved, kept = waits[: len(waits) - keep], waits[len(waits) - keep:]
                si.on_wait = kept
                for w in moved:
                    nop = mybir.InstNoOp(
                        name=f"wsplit-{uid}",
                        engine=inst.engine,
                        bass_nofuse=True,
                        sync_info=mybir.SyncInfo(on_wait=[w], on_update=[]),
                    )
                    uid += 1
                    new.append(nop)
                changed = True
            new.append(inst)
        if changed:
            bb.instructions = new


def _build_nc(reps=1, split_waits=True):
    import contextlib

    import concourse.bass as bass
    import concourse.mybir as mybir
    import concourse.tile as tile
    from concourse.masks import make_identity

    f32 = mybir.dt.float32
    bf16 = mybir.dt.bfloat16
    Alu = mybir.AluOpType
    Act = mybir.ActivationFunctionType

    nc = bass.Bass("TRN2", target_bir_lowering=False, debug=False)
    x_d = nc.dram_tensor("x", [T, D], f32, kind="ExternalInput")
    gwt_d = nc.dram_tensor("gwt", [D, E], f32, kind="ExternalInput")
    # expert weights pre-tiled on host: [ii, d_inner, e, dd, i_inner] so one
    # i-tile's worth of all experts is a single fully-contiguous DMA
    ewt_d = nc.dram_tensor("ewt", [II, P, E, DD, P], bf16, kind="ExternalInput")
    owt_d = nc.dram_tensor("owt", [I, DO], bf16, kind="ExternalInput")
    out_d = nc.dram_tensor("out", [T, DO], f32, kind="ExternalOutput")

    xr = x_d.rearrange("(tt p) d -> p tt d", p=P)
    outr = out_d.rearrange("(tt p) d -> p tt d", p=P)
    gwr = gwt_d.rearrange("(dd p) e -> p dd e", p=P)
    owr = owt_d.rearrange("(ii p) d -> p ii d", p=P)
    ewr = ewt_d

    with tile.TileContext(nc) as tc:
        with (
            tc.tile_pool(name="const", bufs=1) as constp,
            tc.tile_pool(name="xp", bufs=2) as xp,
            tc.tile_pool(name="xtp", bufs=1) as xtp,
            tc.tile_pool(name="gate", bufs=2) as gatep,
            tc.tile_pool(name="x16p", bufs=1) as x16p,
            tc.tile_pool(name="gp", bufs=1) as gp,
            tc.tile_pool(name="zp", bufs=1) as zp,
            tc.tile_pool(name="ewp", bufs=3) as ewp,
            tc.tile_pool(name="hstp", bufs=1) as hstp,
            tc.tile_pool(name="rp", bufs=1) as rp,
            tc.tile_pool(name="obp", bufs=2) as obp,
            tc.tile_pool(name="ps_sm", bufs=3, space="PSUM") as pss,
            tc.tile_pool(name="ps_gate", bufs=1, space="PSUM") as psg,
            tc.tile_pool(name="ps_hs", bufs=2, space="PSUM") as psh,
            tc.tile_pool(name="ps_out", bufs=2, space="PSUM") as pso,
        ):
            ident32 = constp.tile([P, P], f32)
            make_identity(nc, ident32)
            # one-hot rows: onehot8[k, e, :] = (k == e); stationary operand of
            # the rank-1 matmul that broadcasts a mask row to all partitions
            onehot8 = constp.tile([8, E, P], bf16)
            nc.gpsimd.memset(onehot8[:], 0.0)
            nc.gpsimd.affine_select(
                out=onehot8[:], in_=onehot8[:],
                compare_op=mybir.AluOpType.not_equal, fill=1.0, base=0,
                # onehot8[k, e, p] = (k - e != 0) ? 0.0 : 1.0
                pattern=[[-1, E], [0, P]], channel_multiplier=1,
            )
            gw_sb = constp.tile([P, DD, E], f32)
            nc.sync.dma_start(gw_sb[:], gwr[:, :, :])
            # out_w load is emitted late (after phase 1) so the x loads it
            # gates the PE on are not queued behind this 4MB transfer
            ow_sb = constp.tile([P, II, DO], bf16)

            wm_all = constp.tile([P, TT, E], f32)
            t1sq = constp.tile([P, TT], f32)
            xT16 = x16p.tile([P, DD, T], bf16)

            # reps>1 wraps the body in a device-side loop: used only for
            # timing (the body is idempotent), never for grading runs.
            loop_cm = tc.For_i(0, reps, 1) if reps > 1 else contextlib.nullcontext()
            with loop_cm:
                _emit_body(
                    nc, tc, mybir, xr, outr, ewr, owr, gw_sb, ow_sb, ident32,
                    onehot8, wm_all, t1sq, xT16, xp, xtp, gatep, gp, zp,
                    ewp, hstp, rp, obp, pss, psg, psh, pso,
                )
    if split_waits:
        _split_sync_waits(nc)
    return nc


def _emit_body(
    nc, tc, mybir, xr, outr, ewr, owr, gw_sb, ow_sb, ident32,
    onehot8, wm_all, t1sq, xT16, xp, xtp, gatep, gp, zp, ewp, hstp, rp, obp,
    pss, psg, psh, pso,
):
    f32 = mybir.dt.float32
    bf16 = mybir.dt.bfloat16
    Alu = mybir.AluOpType
    Act = mybir.ActivationFunctionType

    if True:  # keep the original indentation structure below
        if True:
            # ---- Phase 1: gating (exact fp32); the x transpose is evicted
            # twice from PSUM: fp32 (gate operand) and bf16 (expert-GEMM
            # operand xT16, kept resident for phase 2)
            for tt in range(TT):
                xt = xp.tile([P, D], f32, tag="xtile")
                nc.sync.dma_start(xt[:], xr[:, tt, :])
                xTt = xtp.tile([P, DD, P], f32, tag="xT")
                for db in range(DD // 4):
                    tp = pss.tile([P, 4 * P], f32, tag="sm")
                    for dq in range(4):
                        dd = db * 4 + dq
                        nc.tensor.transpose(
                            tp[:, dq * P:(dq + 1) * P],
                            xt[:, dd * P:(dd + 1) * P], ident32[:],
                        )
                    nc.vector.tensor_copy(
                        xTt[:, db * 4:(db + 1) * 4, :], tp[:]
                    )
                    for dq in range(4):
                        nc.vector.tensor_copy(
                            xT16[:, db * 4 + dq, tt * P:(tt + 1) * P],
                            tp[:, dq * P:(dq + 1) * P],
                        )
                gps = psg.tile([P, E], f32, tag="gate")
                for dd in range(DD):
                    nc.tensor.matmul(
                        gps[:], xTt[:, dd, :], gw_sb[:, dd, :],
                        start=(dd == 0), stop=(dd == DD - 1),
                    )
                sc = gatep.tile([P, E], f32, tag="sc")
                nc.vector.tensor_copy(sc[:], gps[:])
                s8 = gatep.tile([P, 8], f32, tag="s8")
                nc.vector.max(s8[:], sc[:])
                # split the top-1 weight into sign (exact in bf16, goes into
                # the mask rows) and top1^2 (fp32, applied at the out-proj
                # eviction): relu(w*h)^2 == w^2 * relu(sign(w)*h)^2 exactly.
                sgn = gatep.tile([P, 1], f32, tag="sgn")
                nc.vector.tensor_scalar(
                    sgn[:], s8[:, 0:1], 0.0, None, Alu.is_ge
                )
                nc.vector.tensor_scalar(
                    sgn[:], sgn[:], 2.0, -1.0, Alu.mult, Alu.add
                )
                nc.vector.tensor_scalar(
                    wm_all[:, tt, :], sc[:], s8[:, 3:4], sgn[:, 0:1],
                    Alu.is_ge, Alu.mult,
                )
                nc.vector.tensor_scalar(
                    t1sq[:, tt:tt + 1], s8[:, 0:1], s8[:, 0:1], None, Alu.mult
                )

            # out_w load deferred here so phase-1 x loads are not queued
            # behind it on the DMA queues
            nc.sync.dma_start(ow_sb[:], owr[:, :, :])

            # ---- Phase 2: per token-chunk: masked transpose, expert GEMM,
            #      relu^2, out projection
            for ch in range(NCH):
                # expert-mask rows first: transpose wm [t,e] -> [e,t], then a
                # rank-1 matmul per expert broadcasts the row to all
                # partitions. Emitted before the xT16 transposes so the PE has
                # filler work while DVE drains the broadcast psums.
                wps = psg.tile([P, TC], f32, tag="gate")
                for tl in range(TPC):
                    tt = ch * TPC + tl
                    nc.tensor.transpose(
                        wps[:E, tl * P:(tl + 1) * P], wm_all[:, tt, :],
                        ident32[:],
                    )
                wmT16 = gatep.tile([8, TC], bf16, tag="wmT16")
                nc.vector.tensor_copy(wmT16[:E], wps[:E])
                wrow = gp.tile([P, E, TC], bf16, tag="wrow")
                for e in range(E):
                    bps = pss.tile([P, TC], f32, tag="sm")
                    nc.tensor.matmul(
                        bps[:], onehot8[:, e, :], wmT16[:E],
                        start=True, stop=True,
                    )
                    nc.vector.tensor_copy(wrow[:, e, :], bps[:])
                z16 = zp.tile([P, E, DD, TC], bf16, tag="z16")
                for e in range(E):
                    nc.vector.tensor_tensor(
                        z16[:, e, :, :],
                        xT16[:, :, ch * TC:(ch + 1) * TC],
                        wrow[:, e, None, :].to_broadcast([P, DD, TC]),
                        Alu.mult,
                    )

                hst = hstp.tile([P, II, TC], bf16, tag="hst")
                for ii in range(II):
                    ew = ewp.tile([P, E, DD, P], bf16, tag="ew")
                    nc.sync.dma_start(ew[:], ewr[ii])
                    hps = psh.tile([P, TC], f32, tag="hps")
                    first = True
                    for dd in range(DD):
                        for e in range(E):
                            nc.tensor.matmul(
                                hps[:], ew[:, e, dd, :], z16[:, e, dd, :],
                                start=first,
                                stop=(dd == DD - 1 and e == E - 1),
                            )
                            first = False
                    rt = rp.tile([P, TC], f32, tag="rt")
                    nc.scalar.activation(rt[:], hps[:], Act.Relu)
                    nc.scalar.activation(hst[:, ii, :], rt[:], Act.Square)

                for tl in range(TPC):
                    tt = ch * TPC + tl
                    for dc in range(2):
                        ops = pso.tile([P, 512], f32, tag="ops")
                        for ii in range(II):
                            nc.tensor.matmul(
                                ops[:], hst[:, ii, tl * P:(tl + 1) * P],
                                ow_sb[:, ii, dc * 512:(dc + 1) * 512],
                                start=(ii == 0), stop=(ii == II - 1),
                            )
                        ob = obp.tile([P, 512], f32, tag="ob")
                        nc.vector.tensor_scalar(
                            ob[:], ops[:], t1sq[:, tt:tt + 1], None, Alu.mult
                        )
                        nc.sync.dma_start(
                            outr[:, tt, dc * 512:(dc + 1) * 512], ob[:]
                        )


def _get_nc():
    if "nc" not in _CACHE:
        _CACHE["nc"] = _build_nc()
    return _CACHE["nc"]


def _make_in_maps(inputs):
    x = inputs["x"]
    top_k = int(inputs["top_k"])
    assert top_k == 4, f"kernel hardcodes top_k=4, got {top_k}"
    gate_w, expert_w, out_w = inputs["gate_w"], inputs["expert_w"], inputs["out_w"]
    B, S, Dm = x.shape
    assert (Dm, gate_w.shape[0], expert_w.shape[1], out_w.shape[0]) == (D, E, I, DO)
    xf = np.ascontiguousarray(np.asarray(x, dtype=np.float32).reshape(-1, Dm))
    assert xf.shape[0] == NCORES * T

    bf = ml_dtypes.bfloat16
    gwt = np.ascontiguousarray(np.asarray(gate_w, np.float32).T)           # [D, E]
    # [E, I, D] -> [II, d_inner, E, DD, i_inner] (pre-tiled for contiguous DMA)
    ewt = np.ascontiguousarray(
        np.asarray(expert_w, np.float32)
        .reshape(E, II, P, DD, P)
        .transpose(1, 4, 0, 3, 2)
    ).astype(bf)
    owt = np.ascontiguousarray(np.asarray(out_w, np.float32).T).astype(bf)  # [I, DO]

    return [
        {"x": xf[c * T:(c + 1) * T], "gwt": gwt, "ewt": ewt, "owt": owt}
        for c in range(NCORES)
    ]


def kernel(x, gate_w, expert_w, out_w, top_k):
    from concourse.bass_utils import run_bass_kernel_spmd

    in_maps = _make_in_maps(dict(
        x=x, gate_w=gate_w, expert_w=expert_w, out_w=out_w, top_k=top_k
    ))
    nc = _get_nc()
    res = run_bass_kernel_spmd(nc, in_maps, list(range(NCORES)))
    out = np.concatenate([res.results[c]["out"] for c in range(NCORES)], axis=0)
    B, S, Dm = x.shape
    return out.reshape(B, S, Dm).astype(np.float32)


# revision 37
# speedup vs baseline: 1.0154x; 1.0154x over previous
"""MoE routing kernel for Trainium2 (8 NeuronCores, data-parallel over tokens).

Computation (matches the reference exactly):
    scores = x @ gate_w.T                 [N, E]   fp32 (exact gating!)
    top4 mask per token, weight = top-1 score for all selected experts
    hs = (x * top1) @ sum_{e in top4} expert_w[e].T     (bf16 GEMMs)
    out = relu(hs)^2 @ out_w.T

Sharding: tokens split 1024/core (no collectives). Weights are replicated,
host-side pre-transposed to contraction-major layout and cast to bf16
(gate weights stay fp32 - the top-k selection is discrete and needs exact
scores; bf16 scores would flip expert selections).

Device pipeline per core:
  1. PE fp32 transpose of x tiles -> xT; fp32 gate GEMM -> scores [t,8]
  2. DVE max8 -> sorted scores; mask = (s >= s[3]); x16w = bf16(x * s[0])
  3. "z-trick": for each (token-tile, expert), one bf16 matmul with a
     mask-scaled identity as moving operand produces the masked transpose
     z_e = (x*top1*mask_e)^T. This lets the main expert GEMM accumulate
     over experts AND the contraction dim entirely inside PSUM.
  4. main GEMM: hs^T[i,t] += ewT[e][d,i].T @ z_e[d,t] over (d,e)
  5. relu^2 on ScalarE (fp32 relu, square with bf16 cast)
  6. out projection: out[t,:] = relu2(hs)[t,:] @ out_w.T
"""

import numpy as np
import ml_dtypes

_CACHE = {}

P = 128
T, D, E, I, DO = 1024, 1024, 8, 2048, 1024
TT, DD, II = T // P, D // P, I // P          # 8, 8, 16
NCH = 2                                      # token chunks per core
TPC = TT // NCH                              # t-tiles per chunk (4)
TC = TPC * P                                 # tokens per chunk (512)
NCORES = 8


def _split_sync_waits(nc):
    """walrus in this container caps sync waits per instruction (and rejects
    any wait on Drain). Move excess waits onto injected same-engine NOPs
    placed immediately before the instruction - the engine blocks on the
    nops' waits first, so the ordering semantics are identical."""
    from concourse import mybir

    uid = 0
    for bb in nc.m.functions[0].blocks:
        insts = bb.instructions
        new = []
        changed = False
        for inst in insts:
            si = getattr(inst, "sync_info", None)
            waits = list(si.on_wait) if si is not None and si.on_wait else []
            keep = 0 if isinstance(inst, mybir.InstDrain) else 1
            if len(waits) > keep:
                moved, kept = waits[: len(waits) - keep], waits[len(waits) - keep:]
                si.on_wait = kept
                for w in moved:
                    nop = mybir.InstNoOp(
                        name=f"wsplit-{uid}",
                        engine=inst.engine,
                        bass_nofuse=True,
                        sync_info=mybir.SyncInfo(on_wait=[w], on_update=[]),
                    )
                    uid += 1
                    new.append(nop)
                changed = True
            new.append(inst)
        if changed:
            bb.instructions = new


def _build_nc(reps=1, split_waits=True):
    import contextlib

    import concourse.bass as bass
    import concourse.mybir as mybir
    import concourse.tile as tile
    from concourse.masks import make_identity

    f32 = mybir.dt.float32
    bf16 = mybir.dt.bfloat16
    Alu = mybir.AluOpType
    Act = mybir.ActivationFunctionType

    nc = bass.Bass("TRN2", target_bir_lowering=False, debug=False)
    x_d = nc.dram_tensor("x", [T, D], f32, kind="ExternalInput")
    gwt_d = nc.dram_tensor("gwt", [D, E], f32, kind="ExternalInput")
    # expert weights pre-tiled on host: [ii, d_inner, e, dd, i_inner] so one
    # i-tile's worth of all experts is a single fully-contiguous DMA
    ewt_d = nc.dram_tensor("ewt", [II, P, E, DD, P], bf16, kind="ExternalInput")
    owt_d = nc.dram_tensor("owt", [I, DO], bf16, kind="ExternalInput")
    out_d = nc.dram_tensor("out", [T, DO], f32, kind="ExternalOutput")

    xr = x_d.rearrange("(tt p) d -> p tt d", p=P)
    outr = out_d.rearrange("(tt p) d -> p tt d", p=P)
    gwr = gwt_d.rearrange("(dd p) e -> p dd e", p=P)
    owr = owt_d.rearrange("(ii p) d -> p ii d", p=P)
    ewr = ewt_d

    with tile.TileContext(nc) as tc:
        with (
            tc.tile_pool(name="const", bufs=1) as constp,
            tc.tile_pool(name="xp", bufs=3) as xp,
            tc.tile_pool(name="xtp", bufs=1) as xtp,
            tc.tile_pool(name="gate", bufs=2) as gatep,
            tc.tile_pool(name="x16p", bufs=1) as x16p,
            tc.tile_pool(name="gp", bufs=1) as gp,
            tc.tile_pool(name="zp", bufs=1) as zp,
            tc.tile_pool(name="ewp", bufs=2) as ewp,
            tc.tile_pool(name="hstp", bufs=1) as hstp,
            tc.tile_pool(name="rp", bufs=2) as rp,
            tc.tile_pool(name="obp", bufs=2) as obp,
            tc.tile_pool(name="ps_sm", bufs=3, space="PSUM") as pss,
            tc.tile_pool(name="ps_gate", bufs=1, space="PSUM") as psg,
            tc.tile_pool(name="ps_hs", bufs=2, space="PSUM") as psh,
            tc.tile_pool(name="ps_out", bufs=2, space="PSUM") as pso,
        ):
            ident32 = constp.tile([P, P], f32)
            make_identity(nc, ident32)
            # one-hot rows: onehot8[k, e, :] = (k == e); stationary operand of
            # the rank-1 matmul that broadcasts a mask row to all partitions
            onehot8 = constp.tile([8, E, P], bf16)
            nc.gpsimd.memset(onehot8[:], 0.0)
            nc.gpsimd.affine_select(
                out=onehot8[:], in_=onehot8[:],
                compare_op=mybir.AluOpType.not_equal, fill=1.0, base=0,
                # onehot8[k, e, p] = (k - e != 0) ? 0.0 : 1.0
                pattern=[[-1, E], [0, P]], channel_multiplier=1,
            )
            gw_sb = constp.tile([P, DD, E], f32)
            nc.sync.dma_start(gw_sb[:], gwr[:, :, :])
            # out_w load is emitted late (after phase 1) so the x loads it
            # gates the PE on are not queued behind this 4MB transfer
            ow_sb = constp.tile([P, II, DO], bf16)

            wm_all = constp.tile([P, TT, E], f32)
            t1sq = constp.tile([P, TT], f32)
            xT16 = x16p.tile([P, DD, T], bf16)

            # reps>1 wraps the body in a device-side loop: used only for
            # timing (the body is idempotent), never for grading runs.
            loop_cm = tc.For_i(0, reps, 1) if reps > 1 else contextlib.nullcontext()
            with loop_cm:
                _emit_body(
                    nc, tc, mybir, xr, outr, ewr, owr, gw_sb, ow_sb, ident32,
                    onehot8, wm_all, t1sq, xT16, xp, xtp, gatep, gp, zp,
                    ewp, hstp, rp, obp, pss, psg, psh, pso,
                )
    if split_waits:
        _split_sync_waits(nc)
    return nc


def _emit_body(
    nc, tc, mybir, xr, outr, ewr, owr, gw_sb, ow_sb, ident32,
    onehot8, wm_all, t1sq, xT16, xp, xtp, gatep, gp, zp, ewp, hstp, rp, obp,
    pss, psg, psh, pso,
):
    f32 = mybir.dt.float32
    bf16 = mybir.dt.bfloat16
    Alu = mybir.AluOpType
    Act = mybir.ActivationFunctionType

    if True:  # keep the original indentation structure below
        if True:
            # ---- Phase 1: gating (exact fp32); the x transpose is evicted
            # twice from PSUM: fp32 (gate operand) and bf16 (expert-GEMM
            # operand xT16, kept resident for phase 2)
            for tt in range(TT):
                xt = xp.tile([P, D], f32, tag="xtile")
                nc.sync.dma_start(xt[:], xr[:, tt, :])
                xTt = xtp.tile([P, DD, P], f32, tag="xT")
                for db in range(DD // 4):
                    tp = pss.tile([P, 4 * P], f32, tag="sm")
                    for dq in range(4):
                        dd = db * 4 + dq
                        nc.tensor.transpose(
                            tp[:, dq * P:(dq + 1) * P],
                            xt[:, dd * P:(dd + 1) * P], ident32[:],
                        )
                    nc.vector.tensor_copy(
                        xTt[:, db * 4:(db + 1) * 4, :], tp[:]
                    )
                    for dq in range(4):
                        nc.vector.tensor_copy(
                            xT16[:, db * 4 + dq, tt * P:(tt + 1) * P],
                            tp[:, dq * P:(dq + 1) * P],
                        )
                gps = psg.tile([P, E], f32, tag="gate")
                for dd in range(DD):
                    nc.tensor.matmul(
                        gps[:], xTt[:, dd, :], gw_sb[:, dd, :],
                        start=(dd == 0), stop=(dd == DD - 1),
                    )
                sc = gatep.tile([P, E], f32, tag="sc")
                nc.vector.tensor_copy(sc[:], gps[:])
                s8 = gatep.tile([P, 8], f32, tag="s8")
                nc.vector.max(s8[:], sc[:])
                # split the top-1 weight into sign (exact in bf16, goes into
                # the mask rows) and top1^2 (fp32, applied at the out-proj
                # eviction): relu(w*h)^2 == w^2 * relu(sign(w)*h)^2 exactly.
                sgn = gatep.tile([P, 1], f32, tag="sgn")
                nc.vector.tensor_scalar(
                    sgn[:], s8[:, 0:1], 0.0, None, Alu.is_ge
                )
                nc.vector.tensor_scalar(
                    sgn[:], sgn[:], 2.0, -1.0, Alu.mult, Alu.add
                )
                nc.vector.tensor_scalar(
                    wm_all[:, tt, :], sc[:], s8[:, 3:4], sgn[:, 0:1],
                    Alu.is_ge, Alu.mult,
                )
                nc.vector.tensor_scalar(
                    t1sq[:, tt:tt + 1], s8[:, 0:1], s8[:, 0:1], None, Alu.mult
                )

            # out_w load deferred here so phase-1 x loads are not queued
            # behind it on the DMA queues
            nc.sync.dma_start(ow_sb[:], owr[:, :, :])

            # ---- Phase 2: per token-chunk: masked transpose, expert GEMM,
            #      relu^2, out projection
            for ch in range(NCH):
                # expert-mask rows first: transpose wm [t,e] -> [e,t], then a
                # rank-1 matmul per expert broadcasts the row to all
                # partitions. Emitted before the xT16 transposes so the PE has
                # filler work while DVE drains the broadcast psums.
                wps = psg.tile([P, TC], f32, tag="gate")
                for tl in range(TPC):
                    tt = ch * TPC + tl
                    nc.tensor.transpose(
                        wps[:E, tl * P:(tl + 1) * P], wm_all[:, tt, :],
                        ident32[:],
                    )
                wmT16 = gatep.tile([8, TC], bf16, tag="wmT16")
                nc.vector.tensor_copy(wmT16[:E], wps[:E])
                wrow = gp.tile([P, E, TC], bf16, tag="wrow")
                for e in range(E):
                    bps = pss.tile([P, TC], f32, tag="sm")
                    nc.tensor.matmul(
                        bps[:], onehot8[:, e, :], wmT16[:E],
                        start=True, stop=True,
                    )
                    nc.vector.tensor_copy(wrow[:, e, :], bps[:])
                z16 = zp.tile([P, E, DD, TC], bf16, tag="z16")
                for e in range(E):
                    nc.vector.tensor_tensor(
                        z16[:, e, :, :],
                        xT16[:, :, ch * TC:(ch + 1) * TC],
                        wrow[:, e, None, :].to_broadcast([P, DD, TC]),
                        Alu.mult,
                    )

                hst = hstp.tile([P, II, TC], bf16, tag="hst")
                for ii in range(II):
                    ew = ewp.tile([P, E, DD, P], bf16, tag="ew")
                    nc.sync.dma_start(ew[:], ewr[ii])
                    hps = psh.tile([P, TC], f32, tag="hps")
                    first = True
                    for dd in range(DD):
                        for e in range(E):
                            nc.tensor.matmul(
                                hps[:], ew[:, e, dd, :], z16[:, e, dd, :],
                                start=first,
                                stop=(dd == DD - 1 and e == E - 1),
                            )
                            first = False
                    rt = rp.tile([P, TC], f32, tag="rt")
                    nc.vector.tensor_scalar(rt[:], hps[:], 0.0, None, Alu.max)
                    nc.vector.tensor_tensor(hst[:, ii, :], rt[:], rt[:], Alu.mult)

                for tl in range(TPC):
                    tt = ch * TPC + tl
                    for dc in range(2):
                        ops = pso.tile([P, 512], f32, tag="ops")
                        for ii in range(II):
                            nc.tensor.matmul(
                                ops[:], hst[:, ii, tl * P:(tl + 1) * P],
                                ow_sb[:, ii, dc * 512:(dc + 1) * 512],
                                start=(ii == 0), stop=(ii == II - 1),
                            )
                        ob = obp.tile([P, 512], f32, tag="ob")
                        nc.vector.tensor_scalar(
                            ob[:], ops[:], t1sq[:, tt:tt + 1], None, Alu.mult
                        )
                        nc.sync.dma_start(
                            outr[:, tt, dc * 512:(dc + 1) * 512], ob[:]
                        )


def _get_nc():
    if "nc" not in _CACHE:
        _CACHE["nc"] = _build_nc()
    return _CACHE["nc"]


def _make_in_maps(inputs):
    x = inputs["x"]
    top_k = int(inputs["top_k"])
    assert top_k == 4, f"kernel hardcodes top_k=4, got {top_k}"
    gate_w, expert_w, out_w = inputs["gate_w"], inputs["expert_w"], inputs["out_w"]
    B, S, Dm = x.shape
    assert (Dm, gate_w.shape[0], expert_w.shape[1], out_w.shape[0]) == (D, E, I, DO)
    xf = np.ascontiguousarray(np.asarray(x, dtype=np.float32).reshape(-1, Dm))
    assert xf.shape[0] == NCORES * T

    bf = ml_dtypes.bfloat16
    gwt = np.ascontiguousarray(np.asarray(gate_w, np.float32).T)           # [D, E]
    # [E, I, D] -> [II, d_inner, E, DD, i_inner] (pre-tiled for contiguous DMA)
    ewt = np.ascontiguousarray(
        np.asarray(expert_w, np.float32)
        .reshape(E, II, P, DD, P)
        .transpose(1, 4, 0, 3, 2)
    ).astype(bf)
    owt = np.ascontiguousarray(np.asarray(out_w, np.float32).T).astype(bf)  # [I, DO]

    return [
        {"x": xf[c * T:(c + 1) * T], "gwt": gwt, "ewt": ewt, "owt": owt}
        for c in range(NCORES)
    ]


def kernel(x, gate_w, expert_w, out_w, top_k):
    from concourse.bass_utils import run_bass_kernel_spmd

    in_maps = _make_in_maps(dict(
        x=x, gate_w=gate_w, expert_w=expert_w, out_w=out_w, top_k=top_k
    ))
    nc = _get_nc()
    res = run_bass_kernel_spmd(nc, in_maps, list(range(NCORES)))
    out = np.concatenate([res.results[c]["out"] for c in range(NCORES)], axis=0)
    B, S, Dm = x.shape
    return out.reshape(B, S, Dm).astype(np.float32)
